# revision 10
# baseline (speedup 1.0000x reference)
"""Trainium2 Bass kernel for nn_BaselineGPT (sliding-window GQA attention block).

Sharding: 8 cores = 2 batches x 4 sequence chunks of 512 queries.
Each core computes its 512 output rows end-to-end (QKV proj, RMS norm, RoPE,
windowed GQA attention, output proj).  KV halo of 256 rows comes with the
chunk; chunk 0's missing halo is masked via a -30000 additive score bias
folded into an extra row of K^T.  Pair-head mixing is folded into Wo on the
host (it is linear and applied post-normalization).

v2: fully pipelined rewrite.
 - streaming weight/x loads (compute starts as soon as the first k-tiles land)
 - RMS-norm 1/sqrt via ln/exp (one activation table set, no Sqrt table loads)
 - RoPE as u/w products against [cos|cos], [sin|sin] tables, engine-balanced
   across vector/gpsimd/scalar
 - merged [128,1536] score-psum tiles -> single Exp per (g,qb)
 - per-qb batched softmax denominators: DMA psum rows -> [4,512] reciprocal ->
   gpsimd partition_broadcast (no PE broadcast matmuls)
 - qb-major attention with software-pipelined output projection
"""

import math
from contextlib import ExitStack

import numpy as np

import concourse.bass as bass
from concourse import bacc
import concourse.mybir as mybir
import concourse.tile as tile
from concourse.masks import make_identity

B, S, DIM = 2, 2048, 1024
H, KVH, HD = 16, 4, 64
WINDOW = 256
ROPE_BASE = 10000.0
EPS = 1e-6

NQ = 512          # queries per core
NK = 768          # kv rows per core (incl 256 halo)
NCORES = 8
F32 = mybir.dt.float32
BF16 = mybir.dt.bfloat16
AF = mybir.ActivationFunctionType

_BUILT = None


def _ecopy(eng, nc, out, in_):
    if eng is nc.scalar:
        nc.scalar.copy(out=out, in_=in_)
    else:
        eng.tensor_copy(out=out, in_=in_)


def _build():
    nc = bacc.Bacc(None)

    xt = nc.declare_dram_parameter("xt", [DIM, NK], BF16, isOutput=False)
    wq = nc.declare_dram_parameter("wq", [DIM, DIM], BF16, isOutput=False)
    wkv = nc.declare_dram_parameter("wkv", [DIM, 512], BF16, isOutput=False)
    wo = nc.declare_dram_parameter("wo", [DIM, DIM], BF16, isOutput=False)
    cos2 = nc.declare_dram_parameter("cos2", [NK, HD], BF16, isOutput=False)
    sin2 = nc.declare_dram_parameter("sin2", [NK, HD], BF16, isOutput=False)
    kbias = nc.declare_dram_parameter("kbias", [1, NK], BF16, isOutput=False)
    qgain = nc.declare_dram_parameter("qgain", [1, H], BF16, isOutput=False)
    m0 = nc.declare_dram_parameter("m0", [128, 512], BF16, isOutput=False)
    m2 = nc.declare_dram_parameter("m2", [128, 512], BF16, isOutput=False)
    out = nc.declare_dram_parameter("out", [NQ, DIM], F32, isOutput=True)

    with tile.TileContext(nc) as tc, ExitStack() as ctx:
        const = ctx.enter_context(tc.tile_pool(name="const", bufs=1))
        big = ctx.enter_context(tc.tile_pool(name="big", bufs=1))
        tmp = ctx.enter_context(tc.tile_pool(name="tmp", bufs=4))
        qtg_pool = ctx.enter_context(tc.tile_pool(name="qtg", bufs=16))
        att_pool = ctx.enter_context(tc.tile_pool(name="att", bufs=3))
        yraw_pool = ctx.enter_context(tc.tile_pool(name="yraw", bufs=6))
        nrm_pool = ctx.enter_context(tc.tile_pool(name="nrm", bufs=4))
        tn_pool = ctx.enter_context(tc.tile_pool(name="tn", bufs=5))
        ps_a = ctx.enter_context(tc.tile_pool(name="psa", bufs=2, space="PSUM"))

        # ---- constants / small inputs ----
        ident = const.tile([128, 128], BF16, tag="ident")
        make_identity(nc, ident)
        eps_t = const.tile([128, 1], F32, tag="eps")
        nc.vector.memset(eps_t, EPS)
        qg_sb = const.tile([128, H], BF16, tag="qg")
        nc.gpsimd.dma_start(out=qg_sb, in_=qgain[0:1, :].to_broadcast((128, H)))
        m0_sb = const.tile([128, 512], BF16, tag="m0")
        nc.gpsimd.dma_start(out=m0_sb, in_=m0[:, :])
        m2_sb = const.tile([128, 512], BF16, tag="m2")
        nc.gpsimd.dma_start(out=m2_sb, in_=m2[:, :])
        kb_sb = const.tile([1, NK], BF16, tag="kb")
        nc.gpsimd.dma_start(out=kb_sb, in_=kbias[:, :])
        cos_sb, sin_sb = [], []
        for st in range(6):
            sl = slice(st * 128, st * 128 + 128)
            tc_ = const.tile([128, HD], BF16, tag=f"cos{st}")
            nc.gpsimd.dma_start(out=tc_, in_=cos2[sl, :])
            cos_sb.append(tc_)
            ts_ = const.tile([128, HD], BF16, tag=f"sin{st}")
            nc.gpsimd.dma_start(out=ts_, in_=sin2[sl, :])
            sin_sb.append(ts_)

        # ---- big persistent SBUF tensors; stream x/w loads k-tile-major ----
        xt_sb, wq_sb, wkv_sb = [], [], []
        for kt_ in range(8):
            sl = slice(kt_ * 128, kt_ * 128 + 128)
            t = big.tile([128, NK], BF16, tag=f"xt{kt_}", name=f"xt{kt_}")
            nc.sync.dma_start(out=t[:, 0:384], in_=xt[sl, 0:384])
            nc.sync.dma_start(out=t[:, 384:768], in_=xt[sl, 384:768])
            xt_sb.append(t)
            t = big.tile([128, 512], BF16, tag=f"wkv{kt_}", name=f"wkv{kt_}")
            nc.gpsimd.dma_start(out=t, in_=wkv[sl, :])
            wkv_sb.append(t)
            t = big.tile([128, DIM], BF16, tag=f"wq{kt_}", name=f"wq{kt_}")
            nc.scalar.dma_start(out=t[:, 0:512], in_=wq[sl, 0:512])
            nc.scalar.dma_start(out=t[:, 512:1024], in_=wq[sl, 512:1024])
            wq_sb.append(t)

        q_rope = big.tile([128, 4, DIM], BF16, tag="qrope")
        k_rope = big.tile([128, 6, KVH * HD], BF16, tag="krope")
        v_sb = big.tile([128, 6, KVH, HD + 1], BF16, tag="v")
        kt_sb = big.tile([65, KVH, NK], BF16, tag="kt")
        yt_sb = big.tile([128, 8, NQ], BF16, tag="yt")
        den4 = [
            big.tile([4, 512], BF16, tag=f"den{qb}", name=f"den{qb}")
            for qb in range(4)
        ]

        nc.vector.memset(v_sb[:, :, :, HD : HD + 1], 1.0)
        for g in range(KVH):
            nc.vector.tensor_copy(out=kt_sb[64:65, g, :], in_=kb_sb)

        def rms_rope(psrc, nheads, st, dst, gain):
            """psrc [128, nheads*HD] f32 PSUM -> dst (bf16 slice) with RMS norm,
            optional per-head gain (incl 1/8 scaling), and RoPE at kv tile st."""
            hd2 = HD // 2
            p3 = psrc.rearrange("p (h d) -> p h d", d=HD)
            sq = tmp.tile([128, 16, HD], BF16, tag="sq")
            nc.scalar.activation(out=sq[:, :nheads, :], in_=p3, func=AF.Square)
            ssq = tmp.tile([128, 16], F32, tag="ssq")
            nc.vector.tensor_reduce(
                out=ssq[:, :nheads],
                in_=sq[:, :nheads, :],
                axis=mybir.AxisListType.X,
                op=mybir.AluOpType.add,
            )
            # 1/sqrt(mean+eps) = exp(-0.5*ln(mean+eps)); ln/exp share one
            # activation table set with Exp/Square (no table reloads)
            lt = tmp.tile([128, 16], F32, tag="lt")
            nc.scalar.activation(
                out=lt[:, :nheads], in_=ssq[:, :nheads], func=AF.Ln,
                bias=eps_t, scale=1.0 / HD,
            )
            inv = tmp.tile([128, 16], BF16, tag="inv")
            nc.scalar.activation(
                out=inv[:, :nheads], in_=lt[:, :nheads], func=AF.Exp, scale=-0.5
            )
            if gain:
                nc.vector.tensor_mul(
                    out=inv[:, :nheads], in0=inv[:, :nheads], in1=qg_sb[:, :nheads]
                )
            invb = (
                inv[:, :nheads].rearrange("p (h o) -> p h o", o=1)
                .broadcast_to((128, nheads, HD))
            )
            cosb = (
                cos_sb[st].rearrange("p (o d) -> p o d", o=1)
                .broadcast_to((128, nheads, HD))
            )
            sinb = (
                sin_sb[st].rearrange("p (o d) -> p o d", o=1)
                .broadcast_to((128, nheads, HD))
            )
            cd = tmp.tile([128, 16, HD], BF16, tag="cd")
            nc.vector.tensor_mul(out=cd[:, :nheads, :], in0=cosb, in1=invb)
            sd = tmp.tile([128, 16, HD], BF16, tag="sd")
            nc.gpsimd.tensor_mul(out=sd[:, :nheads, :], in0=sinb, in1=invb)
            # gpsimd cannot read PSUM: stage the projection to SBUF bf16 once
            pb = tmp.tile([128, 16, HD], BF16, tag="pb")
            nc.scalar.copy(out=pb[:, :nheads, :], in_=p3)
            u = tmp.tile([128, 16, HD], BF16, tag="u")
            nc.vector.tensor_mul(
                out=u[:, :nheads, :], in0=pb[:, :nheads, :], in1=cd[:, :nheads, :]
            )
            w = tmp.tile([128, 16, HD], BF16, tag="w")
            nc.gpsimd.tensor_mul(
                out=w[:, :nheads, :], in0=pb[:, :nheads, :], in1=sd[:, :nheads, :]
            )
            dd = dst.rearrange("p (h d) -> p h d", d=HD)
            # o1 = v1*c + v2*s ; o2 = v2*c - v1*s
            nc.vector.tensor_add(
                out=dd[:, :, 0:hd2],
                in0=u[:, :nheads, 0:hd2],
                in1=w[:, :nheads, hd2:HD],
            )
            nc.vector.tensor_sub(
                out=dd[:, :, hd2:HD],
                in0=u[:, :nheads, hd2:HD],
                in1=w[:, :nheads, 0:hd2],
            )

        qtg = {}

        with tc.tile_pool(name="psbt", bufs=2, space="PSUM") as ps_bt:

            def kt_transpose(g, half):
                bt = ps_bt.tile([64, 512], BF16, tag="bt")
                for i in range(3):
                    st = half * 3 + i
                    nc.tensor.transpose(
                        out=bt[:, i * 128 : i * 128 + 128],
                        in_=k_rope[:, st, g * HD : g * HD + HD],
                        identity=ident,
                    )
                eng = nc.vector if g % 2 == 0 else nc.scalar
                _ecopy(eng, nc, kt_sb[0:64, g, half * 384 : half * 384 + 384],
                       bt[:, 0:384])

            def qt_transpose(g, st):
                bt = ps_bt.tile([64, 512], BF16, tag="bt")
                for hh in range(4):
                    h = g * 4 + hh
                    nc.tensor.transpose(
                        out=bt[:, hh * 128 : hh * 128 + 128],
                        in_=q_rope[:, st, h * HD : h * HD + HD],
                        identity=ident,
                    )
                qt = qtg_pool.tile([65, 512], BF16, tag="qtg")
                eng = [nc.scalar, nc.vector, nc.vector, nc.scalar][g]
                _ecopy(eng, nc, qt[0:64, :], bt)
                nc.vector.memset(qt[64:65, :], 1.0)
                qtg[(g, st)] = qt

            # ---- fused K|V projection over 6 kv s-tiles (psum A slices) ----
            pa = None
            for st in range(6):
                if st % 3 == 0:
                    pa = ps_a.tile([128, 1536], F32, tag="psa")
                pkv = pa[:, (st % 3) * 512 : (st % 3) * 512 + 512]
                for kt_ in range(8):
                    nc.tensor.matmul(
                        out=pkv,
                        lhsT=xt_sb[kt_][:, st * 128 : st * 128 + 128],
                        rhs=wkv_sb[kt_],
                        start=(kt_ == 0),
                        stop=(kt_ == 7),
                    )
                nc.scalar.copy(
                    out=v_sb[:, st, :, 0:HD],
                    in_=pkv[:, KVH * HD :].rearrange("p (g d) -> p g d", d=HD),
                )
                rms_rope(pkv[:, 0 : KVH * HD], KVH, st, k_rope[:, st, :], gain=False)

            # ---- Q projection (4 s-tiles x 2 halves) + transposes ----
            qidx = 0
            for st in range(4):
                for half in range(2):
                    if qidx % 3 == 0:
                        pa = ps_a.tile([128, 1536], F32, tag="psa")
                    pq = pa[:, (qidx % 3) * 512 : (qidx % 3) * 512 + 512]
                    qidx += 1
                    for kt_ in range(8):
                        nc.tensor.matmul(
                            out=pq,
                            lhsT=xt_sb[kt_][:, 256 + st * 128 : 384 + st * 128],
                            rhs=wq_sb[kt_][:, half * 512 : half * 512 + 512],
                            start=(kt_ == 0),
                            stop=(kt_ == 7),
                        )
                    rms_rope(
                        pq, 8, st + 2,
                        q_rope[:, st, half * 512 : half * 512 + 512],
                        gain=True,
                    )
                if st == 0:
                    for g in range(KVH):
                        kt_transpose(g, 0)
                elif st == 1:
                    for g in range(KVH):
                        kt_transpose(g, 1)
                else:
                    for g in range(KVH):
                        qt_transpose(g, st - 2)

            # wq/wkv dead after Q proj: wo reuses wq's SBUF slots
            wo_sb = []
            for kt_ in range(8):
                t = big.tile([128, DIM], BF16, tag=f"wq{kt_}", name=f"wo{kt_}")
                sl = slice(kt_ * 128, kt_ * 128 + 128)
                nc.sync.dma_start(out=t[:, 0:512], in_=wo[sl, 0:512])
                nc.sync.dma_start(out=t[:, 512:1024], in_=wo[sl, 512:1024])
                wo_sb.append(t)

            for st in (2, 3):
                for g in range(KVH):
                    qt_transpose(g, st)

        ps_c = ctx.enter_context(tc.tile_pool(name="psc", bufs=2, space="PSUM"))

        # ---- attention, qb-major with software-pipelined outproj ----
        yraws = {}

        def attention(qb):
            dq = den4[qb]
            for g in range(KVH):
                it = g * 4 + qb
                pa = ps_a.tile([128, 1536], F32, tag="psa")
                for t in range(3):
                    nc.tensor.matmul(
                        out=pa[:, t * 512 : t * 512 + 512],
                        lhsT=kt_sb[
                            0:65, g, qb * 128 + t * 128 : qb * 128 + t * 128 + 128
                        ],
                        rhs=qtg[(g, qb)][0:65, :],
                        start=True,
                        stop=True,
                    )
                att = att_pool.tile([128, 1536], BF16, tag="att")
                nc.scalar.activation(out=att, in_=pa, func=AF.Exp)
                nc.vector.tensor_mul(out=att[:, 0:512], in0=att[:, 0:512], in1=m0_sb)
                nc.vector.tensor_mul(
                    out=att[:, 1024:1536], in0=att[:, 1024:1536], in1=m2_sb
                )
                psy = ps_c.tile([128, 512], F32, tag="psc")
                for t in range(3):
                    nc.tensor.matmul(
                        out=psy[0:65, :],
                        lhsT=v_sb[:, qb + t, g, :],
                        rhs=att[:, t * 512 : t * 512 + 512],
                        start=(t == 0),
                        stop=(t == 2),
                    )
                yr = yraw_pool.tile([65, 512], BF16, tag="yraw")
                eng = [nc.scalar, nc.vector, nc.vector, nc.scalar][g]
                _ecopy(eng, nc, yr, psy[0:65, :])
                nc.gpsimd.dma_start(out=dq[g : g + 1, :], in_=yr[64:65, :])
                yraws[it] = yr

        def normalize(qb):
            rec4 = nrm_pool.tile([4, 512], BF16, tag="rec4")
            with nc.allow_low_precision(reason="softmax denom reciprocal in bf16"):
                nc.vector.reciprocal(out=rec4, in_=den4[qb])
            for g in range(KVH):
                it = g * 4 + qb
                # partition_broadcast needs its source at partition 0
                rc1 = nrm_pool.tile([1, 512], BF16, tag="rc1")
                nc.gpsimd.dma_start(out=rc1, in_=rec4[g : g + 1, :])
                rb = nrm_pool.tile([64, 512], BF16, tag="rb")
                nc.gpsimd.partition_broadcast(rb, rc1)
                tn = tn_pool.tile([64, 512], BF16, tag="tn")
                eng = nc.vector if g % 2 == 0 else nc.gpsimd
                eng.tensor_mul(out=tn, in0=yraws[it][0:64, :], in1=rb)
                t3 = tn.rearrange("p (h x) -> p h x", x=128)
                nc.scalar.dma_start(
                    out=yt_sb[0:64, 2 * g : 2 * g + 2, qb * 128 : qb * 128 + 128],
                    in_=t3[:, 0:4:2, :],
                )
                nc.sync.dma_start(
                    out=yt_sb[64:128, 2 * g : 2 * g + 2, qb * 128 : qb * 128 + 128],
                    in_=t3[:, 1:4:2, :],
                )

        def outproj(qb):
            for half in range(2):
                po = ps_c.tile([128, 512], F32, tag="psc")
                for p in range(8):
                    nc.tensor.matmul(
                        out=po,
                        lhsT=yt_sb[:, p, qb * 128 : qb * 128 + 128],
                        rhs=wo_sb[p][:, half * 512 : half * 512 + 512],
                        start=(p == 0),
                        stop=(p == 7),
                    )
                ob = tn_pool.tile([128, 512], F32, tag="ob")
                _ecopy(nc.scalar if half == 0 else nc.vector, nc, ob, po)
                nc.sync.dma_start(
                    out=out[qb * 128 : qb * 128 + 128, half * 512 : half * 512 + 512],
                    in_=ob,
                )

        attention(0)
        normalize(0)
        attention(1)
        normalize(1)
        outproj(0)
        attention(2)
        normalize(2)
        outproj(1)
        attention(3)
        normalize(3)
        outproj(2)
        outproj(3)

    nc.finalize()
    return nc


def _host_inputs(x, Wq, Wk, Wv, Wo, q_gain, pair_mix):
    """Build the 8 per-core input maps."""
    x = np.asarray(x, np.float32)
    Wq = np.asarray(Wq, np.float32)
    Wk = np.asarray(Wk, np.float32)
    Wv = np.asarray(Wv, np.float32)
    Wo = np.asarray(Wo, np.float32)
    q_gain = np.asarray(q_gain, np.float32)
    pair_mix = np.asarray(pair_mix, np.float32)

    # fold pair mixing into Wo:  out = y_mix @ Wo.T,  y_mix = y @ M.T  =>  Wo' = Wo @ M
    M = np.zeros((DIM, DIM), np.float32)
    eye = np.eye(HD, dtype=np.float32)
    for p in range(H // 2):
        for o in range(2):
            for i in range(2):
                ho, hi = 2 * p + o, 2 * p + i
                M[ho * HD : ho * HD + HD, hi * HD : hi * HD + HD] = (
                    pair_mix[p, o, i] * eye
                )
    woT = np.ascontiguousarray((Wo @ M).T)

    wqT = np.ascontiguousarray(Wq.T)
    wkvT = np.ascontiguousarray(np.concatenate([Wk, Wv], axis=0).T)
    qg8 = (q_gain / math.sqrt(HD)).reshape(1, H).astype(np.float32)

    inv_freq = 1.0 / (ROPE_BASE ** (np.arange(0, HD, 2, dtype=np.float32) / HD))

    ql = np.arange(128)
    m0_ = (ql[:, None] >= ql[None, :] + 1).astype(np.float32)  # kl >= ql+1
    m2_ = (ql[:, None] <= ql[None, :]).astype(np.float32)      # kl <= ql
    m0t = np.ascontiguousarray(np.tile(m0_, (1, 4)))
    m2t = np.ascontiguousarray(np.tile(m2_, (1, 4)))

    import ml_dtypes
    bf = ml_dtypes.bfloat16
    wqT, wkvT, woT = (a.astype(bf) for a in (wqT, wkvT, woT))
    m0t, m2t = m0t.astype(bf), m2t.astype(bf)
    qg8 = qg8.astype(bf)
    in_maps = []
    for core in range(NCORES):
        b, c = core // 4, core % 4
        ks = 512 * c - 256
        xc = np.zeros((NK, DIM), np.float32)
        lo = max(0, ks)
        xc[lo - ks :] = x[b, lo : ks + NK]
        t = (ks + np.arange(NK, dtype=np.float32))[:, None]
        freqs = t * inv_freq[None, :]
        kb = np.where(t[:, 0] < 0, -30000.0, 0.0).astype(np.float32).reshape(1, NK)
        cosf = np.cos(freqs).astype(np.float32)
        sinf = np.sin(freqs).astype(np.float32)
        in_maps.append(
            {
                "xt": np.ascontiguousarray(xc.T).astype(bf),
                "wq": wqT,
                "wkv": wkvT,
                "wo": woT,
                "cos2": np.concatenate([cosf, cosf], axis=1).astype(bf),
                "sin2": np.concatenate([sinf, sinf], axis=1).astype(bf),
                "kbias": kb.astype(bf),
                "qgain": qg8,
                "m0": m0t,
                "m2": m2t,
            }
        )
    return in_maps


def kernel(x, Wq, Wk, Wv, Wo, q_gain, pair_mix):
    global _BUILT
    from concourse.bass_utils import run_bass_kernel_spmd

    if _BUILT is None:
        _BUILT = _build()
    in_maps = _host_inputs(x, Wq, Wk, Wv, Wo, q_gain, pair_mix)
    res = run_bass_kernel_spmd(_BUILT, in_maps, list(range(NCORES)))
    out = np.empty((B, S, DIM), np.float32)
    for core in range(NCORES):
        b, c = core // 4, core % 4
        out[b, 512 * c : 512 * c + 512, :] = res.results[core]["out"]
    return out


# revision 11
# speedup vs baseline: 1.7862x; 1.7862x over previous
"""Trainium2 Bass kernel for nn_BaselineGPT (sliding-window GQA attention block).

Sharding: 8 cores = 2 batches x 4 sequence chunks of 512 queries.
Each core computes its 512 output rows end-to-end (QKV proj, RMS norm, RoPE,
windowed GQA attention, output proj).  KV halo of 256 rows comes with the
chunk; chunk 0's missing halo is masked via a -30000 additive score bias
folded into an extra row of K^T.  Pair-head mixing is folded into Wo on the
host (it is linear and applied post-normalization).

v2: fully pipelined rewrite.
 - streaming weight/x loads (compute starts as soon as the first k-tiles land)
 - RMS-norm 1/sqrt via ln/exp (one activation table set, no Sqrt table loads)
 - RoPE as u/w products against [cos|cos], [sin|sin] tables, engine-balanced
   across vector/gpsimd/scalar
 - merged [128,1536] score-psum tiles -> single Exp per (g,qb)
 - per-qb batched softmax denominators: DMA psum rows -> [4,512] reciprocal ->
   gpsimd partition_broadcast (no PE broadcast matmuls)
 - qb-major attention with software-pipelined output projection
"""

import math
from contextlib import ExitStack

import numpy as np

import concourse.bass as bass
from concourse import bacc
import concourse.mybir as mybir
import concourse.tile as tile
from concourse.masks import make_identity

B, S, DIM = 2, 2048, 1024
H, KVH, HD = 16, 4, 64
WINDOW = 256
ROPE_BASE = 10000.0
EPS = 1e-6

NQ = 512          # queries per core
NK = 768          # kv rows per core (incl 256 halo)
NCORES = 8
F32 = mybir.dt.float32
BF16 = mybir.dt.bfloat16
AF = mybir.ActivationFunctionType

_BUILT = None


def _ecopy(eng, nc, out, in_):
    if eng is nc.scalar:
        nc.scalar.copy(out=out, in_=in_)
    else:
        eng.tensor_copy(out=out, in_=in_)


def _build():
    nc = bacc.Bacc(None)

    xt = nc.declare_dram_parameter("xt", [DIM, NK], BF16, isOutput=False)
    wq = nc.declare_dram_parameter("wq", [DIM, DIM], BF16, isOutput=False)
    wkv = nc.declare_dram_parameter("wkv", [DIM, 512], BF16, isOutput=False)
    wo = nc.declare_dram_parameter("wo", [DIM, DIM], BF16, isOutput=False)
    cos2 = nc.declare_dram_parameter("cos2", [NK, HD], BF16, isOutput=False)
    sin2 = nc.declare_dram_parameter("sin2", [NK, HD], BF16, isOutput=False)
    kbias = nc.declare_dram_parameter("kbias", [1, NK], BF16, isOutput=False)
    qgain = nc.declare_dram_parameter("qgain", [1, H], BF16, isOutput=False)
    m0 = nc.declare_dram_parameter("m0", [128, 512], BF16, isOutput=False)
    m2 = nc.declare_dram_parameter("m2", [128, 512], BF16, isOutput=False)
    out = nc.declare_dram_parameter("out", [NQ, DIM], F32, isOutput=True)

    with tile.TileContext(nc) as tc, ExitStack() as ctx:
        const = ctx.enter_context(tc.tile_pool(name="const", bufs=1))
        big = ctx.enter_context(tc.tile_pool(name="big", bufs=1))
        tmp = ctx.enter_context(tc.tile_pool(name="tmp", bufs=4))
        qtg_pool = ctx.enter_context(tc.tile_pool(name="qtg", bufs=16))
        att_pool = ctx.enter_context(tc.tile_pool(name="att", bufs=3))
        yraw_pool = ctx.enter_context(tc.tile_pool(name="yraw", bufs=6))
        nrm_pool = ctx.enter_context(tc.tile_pool(name="nrm", bufs=4))
        tn_pool = ctx.enter_context(tc.tile_pool(name="tn", bufs=5))
        ps_a = ctx.enter_context(tc.tile_pool(name="psa", bufs=2, space="PSUM"))

        # ---- constants / small inputs ----
        ident = const.tile([128, 128], BF16, tag="ident")
        make_identity(nc, ident)
        eps_t = const.tile([128, 1], F32, tag="eps")
        nc.vector.memset(eps_t, EPS)
        qg_sb = const.tile([128, H], BF16, tag="qg")
        m0_sb = const.tile([128, 512], BF16, tag="m0")
        m2_sb = const.tile([128, 512], BF16, tag="m2")
        kb_sb = const.tile([1, NK], BF16, tag="kb")
        ones_sb = const.tile([1, 512], BF16, tag="ones1")
        nc.vector.memset(ones_sb, 1.0)
        cos_sb, sin_sb = [], []
        for st in range(6):
            tc_ = const.tile([128, HD], BF16, tag=f"cos{st}", name=f"cos_t{st}")
            cos_sb.append(tc_)
            ts_ = const.tile([128, HD], BF16, tag=f"sin{st}", name=f"sin_t{st}")
            sin_sb.append(ts_)

        def load_trig(st):
            sl = slice(st * 128, st * 128 + 128)
            nc.gpsimd.dma_start(out=cos_sb[st], in_=cos2[sl, :])
            nc.gpsimd.dma_start(out=sin_sb[st], in_=sin2[sl, :])

        # ---- big persistent SBUF tensors; stream x/w loads k-tile-major ----
        xt_sb, wq_sb, wkv_sb = [], [], []
        for kt_ in range(8):
            sl = slice(kt_ * 128, kt_ * 128 + 128)
            t = big.tile([128, NK], BF16, tag=f"xt{kt_}", name=f"xt{kt_}")
            nc.sync.dma_start(out=t[:, 0:384], in_=xt[sl, 0:384])
            xt_sb.append(t)
            t = big.tile([128, 512], BF16, tag=f"wkv{kt_}", name=f"wkv{kt_}")
            nc.gpsimd.dma_start(out=t, in_=wkv[sl, :])
            wkv_sb.append(t)
            t = big.tile([128, DIM], BF16, tag=f"wq{kt_}", name=f"wq{kt_}")
            nc.scalar.dma_start(out=t[:, 0:512], in_=wq[sl, 0:512])
            nc.scalar.dma_start(out=t[:, 512:1024], in_=wq[sl, 512:1024])
            wq_sb.append(t)
        for kt_ in range(8):
            sl = slice(kt_ * 128, kt_ * 128 + 128)
            nc.sync.dma_start(out=xt_sb[kt_][:, 384:768], in_=xt[sl, 384:768])

        q_rope = big.tile([128, 4, DIM], BF16, tag="qrope")
        k_rope = big.tile([128, 6, KVH * HD], BF16, tag="krope")
        v_sb = big.tile([128, 6, KVH, HD + 1], BF16, tag="v")
        kt_sb = big.tile([65, KVH, NK], BF16, tag="kt")
        yt_sb = big.tile([128, 8, NQ], BF16, tag="yt")
        den4 = [
            big.tile([4, 512], BF16, tag=f"den{qb}", name=f"den{qb}")
            for qb in range(4)
        ]

        nc.vector.memset(v_sb[:, :, :, HD : HD + 1], 1.0)
        nc.gpsimd.dma_start(out=kb_sb, in_=kbias[:, :])
        for g in range(KVH):
            nc.vector.tensor_copy(out=kt_sb[64:65, g, :], in_=kb_sb)

        def rms_rope(psrc, nheads, st, dst, gain):
            """psrc [128, nheads*HD] f32 PSUM -> dst (bf16 slice) with RMS norm,
            optional per-head gain (incl 1/8 scaling), and RoPE at kv tile st."""
            hd2 = HD // 2
            p3 = psrc.rearrange("p (h d) -> p h d", d=HD)
            sq = tmp.tile([128, 16, HD], BF16, tag="sq")
            nc.scalar.activation(out=sq[:, :nheads, :], in_=p3, func=AF.Square)
            ssq = tmp.tile([128, 16], F32, tag="ssq")
            nc.vector.tensor_reduce(
                out=ssq[:, :nheads],
                in_=sq[:, :nheads, :],
                axis=mybir.AxisListType.X,
                op=mybir.AluOpType.add,
            )
            # sqrt(mean+eps) on scalar (Sqrt/Square/Copy share one act table
            # set; Exp gets its own single load at attention start), then the
            # tiny [128,h] reciprocal on vector
            rt = tmp.tile([128, 16], F32, tag="rt")
            nc.scalar.activation(
                out=rt[:, :nheads], in_=ssq[:, :nheads], func=AF.Sqrt,
                bias=eps_t, scale=1.0 / HD,
            )
            inv = tmp.tile([128, 16], BF16, tag="inv")
            with nc.allow_low_precision(reason="rms scale in bf16"):
                nc.vector.reciprocal(out=inv[:, :nheads], in_=rt[:, :nheads])
            if gain:
                nc.vector.tensor_mul(
                    out=inv[:, :nheads], in0=inv[:, :nheads], in1=qg_sb[:, :nheads]
                )
            invb = (
                inv[:, :nheads].rearrange("p (h o) -> p h o", o=1)
                .broadcast_to((128, nheads, HD))
            )
            cosb = (
                cos_sb[st].rearrange("p (o d) -> p o d", o=1)
                .broadcast_to((128, nheads, HD))
            )
            sinb = (
                sin_sb[st].rearrange("p (o d) -> p o d", o=1)
                .broadcast_to((128, nheads, HD))
            )
            cd = tmp.tile([128, 16, HD], BF16, tag="cd")
            nc.vector.tensor_mul(out=cd[:, :nheads, :], in0=cosb, in1=invb)
            sd = tmp.tile([128, 16, HD], BF16, tag="sd")
            nc.gpsimd.tensor_mul(out=sd[:, :nheads, :], in0=sinb, in1=invb)
            # gpsimd cannot read PSUM: stage the projection to SBUF bf16 once
            pb = tmp.tile([128, 16, HD], BF16, tag="pb")
            nc.scalar.copy(out=pb[:, :nheads, :], in_=p3)
            u = tmp.tile([128, 16, HD], BF16, tag="u")
            nc.vector.tensor_mul(
                out=u[:, :nheads, :], in0=pb[:, :nheads, :], in1=cd[:, :nheads, :]
            )
            w = tmp.tile([128, 16, HD], BF16, tag="w")
            weng = nc.gpsimd if nheads == 8 else nc.vector
            weng.tensor_mul(
                out=w[:, :nheads, :], in0=pb[:, :nheads, :], in1=sd[:, :nheads, :]
            )
            dd = dst.rearrange("p (h d) -> p h d", d=HD)
            # o1 = v1*c + v2*s ; o2 = v2*c - v1*s
            nc.vector.tensor_add(
                out=dd[:, :, 0:hd2],
                in0=u[:, :nheads, 0:hd2],
                in1=w[:, :nheads, hd2:HD],
            )
            nc.vector.tensor_sub(
                out=dd[:, :, hd2:HD],
                in0=u[:, :nheads, hd2:HD],
                in1=w[:, :nheads, 0:hd2],
            )

        qtg = {}

        with tc.tile_pool(name="psbt", bufs=2, space="PSUM") as ps_bt:

            def kt_transpose(g, half):
                bt = ps_bt.tile([64, 512], BF16, tag="bt")
                for i in range(3):
                    st = half * 3 + i
                    nc.tensor.transpose(
                        out=bt[:, i * 128 : i * 128 + 128],
                        in_=k_rope[:, st, g * HD : g * HD + HD],
                        identity=ident,
                    )
                eng = nc.vector if g % 2 == 0 else nc.scalar
                _ecopy(eng, nc, kt_sb[0:64, g, half * 384 : half * 384 + 384],
                       bt[:, 0:384])

            def qt_transpose(g, st):
                bt = ps_bt.tile([64, 512], BF16, tag="bt")
                for hh in range(4):
                    h = g * 4 + hh
                    nc.tensor.transpose(
                        out=bt[:, hh * 128 : hh * 128 + 128],
                        in_=q_rope[:, st, h * HD : h * HD + HD],
                        identity=ident,
                    )
                qt = qtg_pool.tile([65, 512], BF16, tag="qtg")
                eng = [nc.scalar, nc.vector, nc.vector, nc.scalar][g]
                _ecopy(eng, nc, qt[0:64, :], bt)
                nc.scalar.dma_start(out=qt[64:65, :], in_=ones_sb)
                qtg[(g, st)] = qt

            # ---- fused K|V projection over 6 kv s-tiles (psum A slices) ----
            pa = None
            for st in range(6):
                load_trig(st)
                if st % 3 == 0:
                    pa = ps_a.tile([128, 1536], F32, tag="psa")
                pkv = pa[:, (st % 3) * 512 : (st % 3) * 512 + 512]
                for kt_ in range(8):
                    nc.tensor.matmul(
                        out=pkv,
                        lhsT=xt_sb[kt_][:, st * 128 : st * 128 + 128],
                        rhs=wkv_sb[kt_],
                        start=(kt_ == 0),
                        stop=(kt_ == 7),
                    )
                nc.scalar.copy(
                    out=v_sb[:, st, :, 0:HD],
                    in_=pkv[:, KVH * HD :].rearrange("p (g d) -> p g d", d=HD),
                )
                rms_rope(pkv[:, 0 : KVH * HD], KVH, st, k_rope[:, st, :], gain=False)

            # ---- Q projection (4 s-tiles x 2 halves) + transposes ----
            nc.gpsimd.dma_start(out=m0_sb, in_=m0[:, :])
            nc.gpsimd.dma_start(out=m2_sb, in_=m2[:, :])
            nc.gpsimd.dma_start(out=qg_sb, in_=qgain[0:1, :].to_broadcast((128, H)))
            qidx = 0
            for st in range(4):
                for half in range(2):
                    if qidx % 3 == 0:
                        pa = ps_a.tile([128, 1536], F32, tag="psa")
                    pq = pa[:, (qidx % 3) * 512 : (qidx % 3) * 512 + 512]
                    qidx += 1
                    for kt_ in range(8):
                        nc.tensor.matmul(
                            out=pq,
                            lhsT=xt_sb[kt_][:, 256 + st * 128 : 384 + st * 128],
                            rhs=wq_sb[kt_][:, half * 512 : half * 512 + 512],
                            start=(kt_ == 0),
                            stop=(kt_ == 7),
                        )
                    rms_rope(
                        pq, 8, st + 2,
                        q_rope[:, st, half * 512 : half * 512 + 512],
                        gain=True,
                    )
                if st == 0:
                    for g in range(KVH):
                        kt_transpose(g, 0)
                elif st == 1:
                    for g in range(KVH):
                        kt_transpose(g, 1)
                else:
                    for g in range(KVH):
                        qt_transpose(g, st - 2)

            # wq/wkv dead after Q proj: wo reuses wq's SBUF slots
            wo_sb = []
            for kt_ in range(8):
                t = big.tile([128, DIM], BF16, tag=f"wq{kt_}", name=f"wo{kt_}")
                sl = slice(kt_ * 128, kt_ * 128 + 128)
                nc.sync.dma_start(out=t[:, 0:512], in_=wo[sl, 0:512])
                nc.sync.dma_start(out=t[:, 512:1024], in_=wo[sl, 512:1024])
                wo_sb.append(t)

            for st in (2, 3):
                for g in range(KVH):
                    qt_transpose(g, st)

        ps_c = ctx.enter_context(tc.tile_pool(name="psc", bufs=2, space="PSUM"))

        # ---- attention, qb-major with software-pipelined outproj ----
        yraws = {}

        def attention(qb):
            dq = den4[qb]
            for g in range(KVH):
                it = g * 4 + qb
                pa = ps_a.tile([128, 1536], F32, tag="psa")
                for t in range(3):
                    nc.tensor.matmul(
                        out=pa[:, t * 512 : t * 512 + 512],
                        lhsT=kt_sb[
                            0:65, g, qb * 128 + t * 128 : qb * 128 + t * 128 + 128
                        ],
                        rhs=qtg[(g, qb)][0:65, :],
                        start=True,
                        stop=True,
                    )
                att = att_pool.tile([128, 1536], BF16, tag="att")
                nc.scalar.activation(out=att, in_=pa, func=AF.Exp)
                nc.vector.tensor_mul(out=att[:, 0:512], in0=att[:, 0:512], in1=m0_sb)
                nc.vector.tensor_mul(
                    out=att[:, 1024:1536], in0=att[:, 1024:1536], in1=m2_sb
                )
                psy = ps_c.tile([128, 512], F32, tag="psc")
                for t in range(3):
                    nc.tensor.matmul(
                        out=psy[0:65, :],
                        lhsT=v_sb[:, qb + t, g, :],
                        rhs=att[:, t * 512 : t * 512 + 512],
                        start=(t == 0),
                        stop=(t == 2),
                    )
                yr = yraw_pool.tile([65, 512], BF16, tag="yraw")
                eng = [nc.scalar, nc.vector, nc.vector, nc.scalar][g]
                _ecopy(eng, nc, yr, psy[0:65, :])
                nc.sync.dma_start(out=dq[g : g + 1, :], in_=yr[64:65, :])
                yraws[it] = yr

        def normalize(qb):
            d4f = nrm_pool.tile([4, 512], F32, tag="d4f")
            nc.vector.tensor_copy(out=d4f, in_=den4[qb])
            r4f = nrm_pool.tile([4, 512], F32, tag="r4f")
            nc.vector.reciprocal_approx_fast(out=r4f, in_=d4f)
            rec4 = nrm_pool.tile([4, 512], BF16, tag="rec4")
            nc.vector.tensor_copy(out=rec4, in_=r4f)
            for g in range(KVH):
                it = g * 4 + qb
                # partition_broadcast needs its source at partition 0
                rc1 = nrm_pool.tile([1, 512], BF16, tag="rc1")
                nc.sync.dma_start(out=rc1, in_=rec4[g : g + 1, :])
                rb = nrm_pool.tile([64, 512], BF16, tag="rb")
                nc.gpsimd.partition_broadcast(rb, rc1)
                tn = tn_pool.tile([64, 512], BF16, tag="tn")
                nc.vector.tensor_mul(out=tn, in0=yraws[it][0:64, :], in1=rb)
                t3 = tn.rearrange("p (h x) -> p h x", x=128)
                nc.scalar.dma_start(
                    out=yt_sb[0:64, 2 * g : 2 * g + 2, qb * 128 : qb * 128 + 128],
                    in_=t3[:, 0:4:2, :],
                )
                nc.sync.dma_start(
                    out=yt_sb[64:128, 2 * g : 2 * g + 2, qb * 128 : qb * 128 + 128],
                    in_=t3[:, 1:4:2, :],
                )

        def outproj(qb):
            for half in range(2):
                po = ps_c.tile([128, 512], F32, tag="psc")
                for p in range(8):
                    nc.tensor.matmul(
                        out=po,
                        lhsT=yt_sb[:, p, qb * 128 : qb * 128 + 128],
                        rhs=wo_sb[p][:, half * 512 : half * 512 + 512],
                        start=(p == 0),
                        stop=(p == 7),
                    )
                ob = tn_pool.tile([128, 512], F32, tag="ob")
                _ecopy(nc.scalar if half == 0 else nc.vector, nc, ob, po)
                nc.sync.dma_start(
                    out=out[qb * 128 : qb * 128 + 128, half * 512 : half * 512 + 512],
                    in_=ob,
                )

        attention(0)
        attention(1)
        normalize(0)
        attention(2)
        outproj(0)
        normalize(1)
        attention(3)
        outproj(1)
        normalize(2)
        outproj(2)
        normalize(3)
        outproj(3)

    nc.finalize()
    return nc


def _host_inputs(x, Wq, Wk, Wv, Wo, q_gain, pair_mix):
    """Build the 8 per-core input maps."""
    x = np.asarray(x, np.float32)
    Wq = np.asarray(Wq, np.float32)
    Wk = np.asarray(Wk, np.float32)
    Wv = np.asarray(Wv, np.float32)
    Wo = np.asarray(Wo, np.float32)
    q_gain = np.asarray(q_gain, np.float32)
    pair_mix = np.asarray(pair_mix, np.float32)

    # fold pair mixing into Wo:  out = y_mix @ Wo.T,  y_mix = y @ M.T  =>  Wo' = Wo @ M
    M = np.zeros((DIM, DIM), np.float32)
    eye = np.eye(HD, dtype=np.float32)
    for p in range(H // 2):
        for o in range(2):
            for i in range(2):
                ho, hi = 2 * p + o, 2 * p + i
                M[ho * HD : ho * HD + HD, hi * HD : hi * HD + HD] = (
                    pair_mix[p, o, i] * eye
                )
    woT = np.ascontiguousarray((Wo @ M).T)

    wqT = np.ascontiguousarray(Wq.T)
    wkvT = np.ascontiguousarray(np.concatenate([Wk, Wv], axis=0).T)
    qg8 = (q_gain / math.sqrt(HD)).reshape(1, H).astype(np.float32)

    inv_freq = 1.0 / (ROPE_BASE ** (np.arange(0, HD, 2, dtype=np.float32) / HD))

    ql = np.arange(128)
    m0_ = (ql[:, None] >= ql[None, :] + 1).astype(np.float32)  # kl >= ql+1
    m2_ = (ql[:, None] <= ql[None, :]).astype(np.float32)      # kl <= ql
    m0t = np.ascontiguousarray(np.tile(m0_, (1, 4)))
    m2t = np.ascontiguousarray(np.tile(m2_, (1, 4)))

    import ml_dtypes
    bf = ml_dtypes.bfloat16
    wqT, wkvT, woT = (a.astype(bf) for a in (wqT, wkvT, woT))
    m0t, m2t = m0t.astype(bf), m2t.astype(bf)
    qg8 = qg8.astype(bf)
    in_maps = []
    for core in range(NCORES):
        b, c = core // 4, core % 4
        ks = 512 * c - 256
        xc = np.zeros((NK, DIM), np.float32)
        lo = max(0, ks)
        xc[lo - ks :] = x[b, lo : ks + NK]
        t = (ks + np.arange(NK, dtype=np.float32))[:, None]
        freqs = t * inv_freq[None, :]
        kb = np.where(t[:, 0] < 0, -30000.0, 0.0).astype(np.float32).reshape(1, NK)
        cosf = np.cos(freqs).astype(np.float32)
        sinf = np.sin(freqs).astype(np.float32)
        in_maps.append(
            {
                "xt": np.ascontiguousarray(xc.T).astype(bf),
                "wq": wqT,
                "wkv": wkvT,
                "wo": woT,
                "cos2": np.concatenate([cosf, cosf], axis=1).astype(bf),
                "sin2": np.concatenate([sinf, sinf], axis=1).astype(bf),
                "kbias": kb.astype(bf),
                "qgain": qg8,
                "m0": m0t,
                "m2": m2t,
            }
        )
    return in_maps


def kernel(x, Wq, Wk, Wv, Wo, q_gain, pair_mix):
    global _BUILT
    from concourse.bass_utils import run_bass_kernel_spmd

    if _BUILT is None:
        _BUILT = _build()
    in_maps = _host_inputs(x, Wq, Wk, Wv, Wo, q_gain, pair_mix)
    res = run_bass_kernel_spmd(_BUILT, in_maps, list(range(NCORES)))
    out = np.empty((B, S, DIM), np.float32)
    for core in range(NCORES):
        b, c = core // 4, core % 4
        out[b, 512 * c : 512 * c + 512, :] = res.results[core]["out"]
    return out
